# revision 34
# baseline (speedup 1.0000x reference)
"""4D circular cross-correlation (qcd_ml C_Convolution, k=3, nd=4) on 8 TRN2 cores.

Math: out[o, x,y,z,t, s,c] = b[o] + sum_{i, ax,ay,az,at} W[i,o,ax,ay,az,at]
                                   * U[i, x+ax-1, y+ay-1, z+az-1, t+at-1, s,c]
(all site indices circular). U complex64 (4,16,16,16,32,4,3), W complex64
(4,4,3,3,3,3), b complex64 (4,).

Device mapping (per core, T sharded 8-way with +-1 halos prepared on host):
  - contraction (matmul partition) dim = (reim_in 2, C_in 4, X 16) = 128
  - output (PSUM partition) dim       = (reim_out 2, C_out 4, X0 16) = 128
  - X offsets (ax) live inside the stationary 128x128 matrices, circularly
    banded in (x, x0); complex arithmetic is the 2x2 [[Wr, Wi], [-Wi, Wr]]
    block over the reim axes.
  - The T offsets (at) are removed by a host-side Winograd F(4,3) transform
    along t: the 4 local t outputs form ONE tile whose 6-point input window
    is exactly the t-halo slab; U becomes 6 phases (B^T d), weights become
    G W (6 phases); the device accumulates 9 (ay,az) offsets per phase into
    PSUM and combines the 6 phase results with A^T on the vector engine.
  - moving free dim = (y-pair 2, z 16, spin*color 12) = 384 <= 512 (one
    PSUM bank per phase).
  - y,z circular handled by host padding to 18; t halo from neighbor T-slab.
"""

import os
import sys
import itertools
import numpy as np

for _p in ("/opt/trn_rl_repo",):
    if _p not in sys.path and os.path.isdir(_p):
        sys.path.insert(0, _p)

C_IN, C_OUT = 4, 4
X = Y = Z = 16
T = 32
SC = 12  # spin*color
NCORES = 8
TLOC = T // NCORES          # 4 = one F(4,3) output tile
NPH = 6                     # Winograd F(4,3) phases
YPAD, ZPAD = Y + 2, Z + 2   # 18
UH_ROWS = 10                # y_pad rows per half tile (0..9 / 8..17)
OFF9 = list(itertools.product(range(3), repeat=2))  # (ay, az)
FREE = 2 * Z * SC           # 384, one chunk = (y-pair, z, sc) per phase

USE_FP32R = os.environ.get("CONV_FP32R", "1") == "1"

# Winograd F(4,3), points [0,1,-1,2,-2,inf] (correlation form:
# out[r] = sum_k g[k] d[r+k], r=0..3, d = U[t0-1 .. t0+4]).
BT = np.array([
    [4, 0, -5, 0, 1, 0],
    [0, -4, -4, 1, 1, 0],
    [0, 4, -4, -1, 1, 0],
    [0, -2, -1, 2, 1, 0],
    [0, 2, -1, -2, 1, 0],
    [0, 4, 0, -5, 0, 1]], np.float64)
G = np.array([
    [1 / 4, 0, 0],
    [-1 / 6, -1 / 6, -1 / 6],
    [-1 / 6, 1 / 6, -1 / 6],
    [1 / 24, 1 / 12, 1 / 6],
    [1 / 24, -1 / 12, 1 / 6],
    [0, 0, 1]], np.float64)
# A^T = [[1,1,1,1,1,0],
#        [0,1,-1,2,-2,0],
#        [0,1,1,4,4,0],
#        [0,1,-1,8,-8,1]]  -- applied on the device (DVE).


def _prep_u_shards(U):
    """U complex (4,16,16,16,32,4,3) -> per-core float32 arrays
    [128, YPAD, ZPAD, NPH, SC] of the t-Winograd-transformed field."""
    Ur = np.stack([U.real, U.imag], axis=0).astype(np.float32)  # (2,4,X,Y,Z,T,4,3)
    Ur = Ur.reshape(2, C_IN, X, Y, Z, T, SC)
    Up = np.pad(Ur, ((0, 0), (0, 0), (0, 0), (1, 1), (1, 1), (0, 0), (0, 0)),
                mode="wrap")  # (2,4,16,18,18,32,12)
    shards = []
    for k in range(NCORES):
        t0 = k * TLOC
        tidx = np.arange(t0 - 1, t0 + 5) % T        # 6-point window
        d = np.take(Up, tidx, axis=5)               # (2,4,16,18,18,6,12)
        m = np.einsum("pk,rixyzks->rixyzps", BT,
                      d.astype(np.float64)).astype(np.float32)
        m = m.reshape(128, YPAD, ZPAD, NPH, SC)
        shards.append(np.ascontiguousarray(m))
    return shards


def _prep_wstat(W):
    """W complex (4,4,3,3,3,3) -> [128, NPH*9, 128] float32 stationary stack.

    For phase p and (ay,az): Wg[p][i,o,ax,ay,az] = sum_at G[p,at] W[..,at];
    band in (x,x0): ax = (x - x0 + 1) mod 16 in {0,1,2};
    ri block M = [[Wr, Wi], [-Wi, Wr]] (columns riO: out_r, out_i).
    """
    Wc = np.ascontiguousarray(W).astype(np.complex128)
    Wg = np.einsum("pk,ioxyzk->pioxyz", G.astype(np.complex128), Wc)
    Wg = Wg.astype(np.complex64)                    # (6,4,4,3,3,3)
    stat = np.zeros((2, C_IN, X, NPH * 9, 2, C_OUT, X), np.float32)
    for ph in range(NPH):
        for aidx, (ay, az) in enumerate(OFF9):
            widx = ph * 9 + aidx
            for ax in range(3):
                wr = Wg[ph, :, :, ax, ay, az].real
                wi = Wg[ph, :, :, ax, ay, az].imag
                for x0 in range(X):
                    x = (x0 + ax - 1) % X
                    stat[0, :, x, widx, 0, :, x0] = wr
                    stat[1, :, x, widx, 0, :, x0] = -wi
                    stat[0, :, x, widx, 1, :, x0] = wi
                    stat[1, :, x, widx, 1, :, x0] = wr
    return np.ascontiguousarray(stat.reshape(128, NPH * 9, 128))


def _assemble(results, b):
    """results[k]["out"]: [128, Y, Z, TLOC, SC] f32 -> complex (4,16,16,16,32,4,3)."""
    out = np.empty((C_OUT, X, Y, Z, T, SC), np.complex64)
    for k in range(NCORES):
        r = np.asarray(results[k]["out"], np.float32).reshape(2, C_OUT, X, Y, Z, TLOC, SC)
        out[:, :, :, :, k * TLOC:(k + 1) * TLOC, :] = r[0] + 1j * r[1]
    out += np.asarray(b, np.complex64).reshape(C_OUT, 1, 1, 1, 1, 1)
    return np.ascontiguousarray(out.reshape(C_OUT, X, Y, Z, T, 4, 3))


def _build_nc():
    import concourse.mybir as mybir
    from concourse import bacc, tile
    from contextlib import ExitStack

    f32 = mybir.dt.float32
    mm_dt = mybir.dt.float32r if USE_FP32R else f32
    AluOp = mybir.AluOpType

    WCOLS = NPH * 9 * 128              # 6912
    UCOLS = UH_ROWS * ZPAD * NPH * SC  # 12960

    nc = bacc.Bacc()
    # Fine-grained consumption-ordered input streaming: one full U~ tile
    # filled by disjoint row-slice DMAs (no y duplication), wstat split per
    # phase. Pair 0's phase-0 data (ws[0] + rows 0..5 of phase 0) lands after
    # ~1.2 MB, so the PE starts within a few us; dependencies are tracked at
    # address level, so each matmul only waits for the slices it reads.
    w_dram = nc.declare_dram_parameter("wstat", [128, NPH, 9, 128], mm_dt, isOutput=False)
    u_dram = nc.declare_dram_parameter("u", [128, YPAD, ZPAD, NPH, SC], mm_dt, isOutput=False)
    o_dram = nc.declare_dram_parameter("out", [128, Y, Z, TLOC, SC], f32, isOutput=True)

    with tile.TileContext(nc) as tc, ExitStack() as ctx:
        ipool = ctx.enter_context(tc.tile_pool(name="inp", bufs=1))
        opool = ctx.enter_context(tc.tile_pool(name="o", bufs=2))
        tpool = ctx.enter_context(tc.tile_pool(name="tmp", bufs=1))
        ppool = ctx.enter_context(tc.tile_pool(name="psum", bufs=8, space="PSUM"))

        wt = ipool.tile([128, NPH, 9, 128], mm_dt, tag="w")
        ufull = ipool.tile([128, YPAD, ZPAD, NPH, SC], mm_dt, tag="u")
        # All slices are CONTIGUOUS per partition (phase-strided DMAs measure
        # ~2x slower). Order: wstat ph 0-2, U rows for pair 0, wstat ph 3-5,
        # then the remaining row slices stream in under compute.
        nc.sync.dma_start(wt[:, 0:1], w_dram[:, 0:1])
        nc.sync.dma_start(ufull[:, 0:3], u_dram[:, 0:3])
        nc.sync.dma_start(ufull[:, 3:4], u_dram[:, 3:4])
        nc.sync.dma_start(wt[:, 1:3], w_dram[:, 1:3])
        nc.sync.dma_start(wt[:, 3:6], w_dram[:, 3:6])
        for r0, r1 in ((4, 6), (6, 10), (10, 14), (14, 18)):
            nc.sync.dma_start(ufull[:, r0:r1], u_dram[:, r0:r1])

        def stt(out_ap, sb_in, scalar, ps_or_sb):
            # out = (sb_in * scalar) +/- second operand, via scalar_tensor_tensor
            nc.vector.scalar_tensor_tensor(
                out_ap, in0=sb_in, scalar=scalar, in1=ps_or_sb,
                op0=AluOp.mult, op1=AluOp.add)

        YG = 2  # one out-DMA per y-pair: short tail, early PSUM drain
        for g in range(Y // YG):
            ot = opool.tile([128, YG, Z, TLOC, SC], f32)
            for pair in range(YG // 2):
                y = g * YG + pair * 2               # even; pair (y, y+1)
                pts = []
                for ph in range(NPH):
                    pt = ppool.tile([128, FREE], f32)
                    for aidx, (ay, az) in enumerate(OFF9):
                        rhs = ufull[:, y + ay: y + ay + 2, az: az + Z, ph, :]
                        nc.tensor.matmul(
                            pt[:],
                            wt[:, ph, aidx, :],
                            rhs,
                            start=(aidx == 0),
                            stop=(aidx == 8),
                        )
                    pts.append(pt)
                # A^T combine; every DVE op reads at most one PSUM operand.
                # b=m1+m2, a=m1-m2, u=m3+m4, s=m3-m4
                # t0=m0+b+u; t1=a+2s; t2=b+4u; t3=a+8s+m5
                # Ordered so PSUM banks m1,m2,m0,m3,m4 free as early as
                # possible (the next pair's matmuls reuse them).
                ov = ot[:, pair * 2: pair * 2 + 2]  # [128, 2, Z, TLOC, SC]
                m1c = tpool.tile([128, FREE], f32, tag="m1c")
                nc.vector.tensor_copy(m1c[:], pts[1][:])
                bt_ = tpool.tile([128, FREE], f32, tag="bt")
                nc.vector.tensor_add(bt_[:], m1c[:], pts[2][:])
                t0a = tpool.tile([128, FREE], f32, tag="t0a")
                nc.vector.tensor_add(t0a[:], bt_[:], pts[0][:])
                m3c = tpool.tile([128, FREE], f32, tag="m3c")
                nc.vector.tensor_copy(m3c[:], pts[3][:])
                ut_ = tpool.tile([128, FREE], f32, tag="ut")
                nc.vector.tensor_add(ut_[:], m3c[:], pts[4][:])
                a_ = tpool.tile([128, FREE], f32, tag="at")
                nc.vector.scalar_tensor_tensor(
                    a_[:], in0=m1c[:], scalar=2.0, in1=bt_[:],
                    op0=AluOp.mult, op1=AluOp.subtract)
                s_ = tpool.tile([128, FREE], f32, tag="st")
                nc.vector.scalar_tensor_tensor(
                    s_[:], in0=m3c[:], scalar=2.0, in1=ut_[:],
                    op0=AluOp.mult, op1=AluOp.subtract)
                t3a = tpool.tile([128, FREE], f32, tag="t3a")
                nc.vector.scalar_tensor_tensor(
                    t3a[:], in0=s_[:], scalar=8.0, in1=a_[:],
                    op0=AluOp.mult, op1=AluOp.add)
                # writes into ot: view dims (y2, z, sc) at fixed t=r
                def ow(r):
                    return ov[:, :, :, r, :]
                nc.vector.tensor_add(ow(0), t0a[:], ut_[:])
                stt(ow(1), s_[:], 2.0, a_[:])
                stt(ow(2), ut_[:], 4.0, bt_[:])
                nc.vector.tensor_add(ow(3), t3a[:], pts[5][:])
            nc.sync.dma_start(o_dram[:, g * YG:(g + 1) * YG], ot[:])

    # Bacc defers register allocation and sync-wait splitting to finalize();
    # run_bass_via_pjrt serializes the module as-is, so finalize here.
    nc.finalize()
    return nc


_NC_CACHE = None
LAST_RUN = None  # BassKernelResults of the most recent device run (for test.py)


def kernel(U, W, b):
    global _NC_CACHE, LAST_RUN
    shards = _prep_u_shards(np.asarray(U))
    wstat = _prep_wstat(np.asarray(W))

    if os.environ.get("CONV_EMULATE", "0") == "1":
        results = _emulate(shards, wstat)
    else:
        from concourse.bass_utils import run_bass_kernel_spmd
        if _NC_CACHE is None:
            _NC_CACHE = _build_nc()
        wr = np.ascontiguousarray(wstat.reshape(128, NPH, 9, 128))
        in_maps = [{"wstat": wr, "u": u} for u in shards]
        trace = os.environ.get("CONV_TRACE", "0") == "1"
        LAST_RUN = run_bass_kernel_spmd(
            _NC_CACHE, in_maps, core_ids=list(range(NCORES)), trace=trace)
        results = LAST_RUN.results
    return _assemble(results, np.asarray(b))


def _emulate(shards, wstat):
    """Host-side emulation of the device program (float64 accumulate)."""
    AT = np.array([
        [1, 1, 1, 1, 1, 0],
        [0, 1, -1, 2, -2, 0],
        [0, 1, 1, 4, 4, 0],
        [0, 1, -1, 8, -8, 1]], np.float64)
    results = []
    for u in shards:
        out = np.zeros((128, Y, Z, TLOC, SC), np.float64)
        for y in range(0, Y, 2):
            ms = []
            for ph in range(NPH):
                acc = np.zeros((128, FREE), np.float64)
                for aidx, (ay, az) in enumerate(OFF9):
                    slab = u[:, y + ay: y + ay + 2, az:az + Z, ph, :].reshape(128, -1)
                    acc += wstat[:, ph * 9 + aidx, :].T.astype(np.float64) @ slab.astype(np.float64)
                ms.append(acc.reshape(128, 2, Z, SC))
            m = np.stack(ms, axis=0)  # (6, 128, 2, Z, SC)
            res = np.einsum("rp,pnyzs->nyzrs", AT, m)  # (128, 2, Z, 4, SC)
            out[:, y:y + 2, :, :, :] = res
        results.append({"out": out.reshape(128, Y, Z, TLOC, SC)})
    return results
